# revision 1
# baseline (speedup 1.0000x reference)
"""Trainium2 Bass kernel for CrossScaleAttention.

Computes, for input x [B=8, C=256, H=48, W=48] (N = H*W = 2304):
    q = Wq x + bq ; k = Wk x + bk ; v = Wv x + bv       (1x1 conv projections)
    per head h (4 heads, d=64): attn = softmax(q_h^T k_h / 8)
    o_h = v_h attn^T ; out = Wo o + bo ; y = x + gamma * out

Sharding: data-parallel over batch; core b handles batch element b.
No collectives; each core loads its slice + replicated weights and
writes its output slice.

Device algorithm per core (all matmuls fp32r: 1 col/cycle at N>=256):
  - Q, K in native [o, n] layout:  Q = WqT^T @ X   (lhsT = Wq^T chunks)
  - V^T directly via  V1T = X^T @ WvT  (lhsT = X chunks) with a ones
    column per head (cols h*65+64, DMA'd once) so the attention A@V
    matmul also produces the softmax row-sums for free.
  - Scores computed TRANSPOSED (S^T[m, n] = k^T q) so no PE transposes
    are needed anywhere: lhsT = k chunk [64, 128], rhs = q [64, nb].
  - exp(S^T/8) fused into the PSUM->SBUF evacuation on the scalar
    engine, 4 m-chunks per ACT op (softmax without max-subtraction:
    scores are ~N(0,1), safely inside fp32 exp range for these inputs).
  - AV: psum[65, nb] += V1T_chunk[128, 65]^T @ E^T_chunk[128, nb] over
    18 m-chunks; row 64 = softmax denominators.
  - normalize: row-copy sums, K=1 PE broadcast to 64 partitions,
    reciprocal (fused PSUM->SBUF), multiply on DVE while evacuating.
  - O projection with K=64 per-head chunks of Wo^T; residual fused on
    DVE: y = proj * gamma + x  (+ bias terms when nonzero).

Instruction count is minimized aggressively (packed weight loads, wide
grouped ACT ops, paired PSUM evacuations): the measured runtime in this
environment scales with instruction count.

All tiles that feed PE matmuls are declared float32r (the BIR verifier
requires fp32r operands to be produced as fp32r); DMA loads bitcast the
f32 DRAM side, and compute producers write with fp32r output dtype.
"""

import numpy as np

import concourse.bass as bass
import concourse.mybir as mybir
import concourse.tile as tile

F32 = mybir.dt.float32
F32R = mybir.dt.float32r
AF = mybir.ActivationFunctionType

C = 256
N = 2304  # 48*48
NH = 4
HD = 64  # head dim
HD1 = HD + 1
KC = 128  # contraction chunk
NMC = N // KC  # 18 m-chunks
BLOCKS = [(0, 512), (512, 512), (1024, 512), (1536, 512), (2048, 256)]
GRP = 4  # m-chunks per exp ACT op (4-bank PSUM tile)
GRPS = [4, 4, 4, 4, 2]

_MAX_WAITS = 1  # walrus in this environment accepts 1 sync-wait per instruction


def _split_multi_waits(nc):
    """Hoist excess sem-waits onto same-engine NoOps emitted just before the
    owning instruction (the engine stalls at the NoOp instead — identical
    semantics, one wait per instruction)."""
    n = 0
    for bb in nc.m.functions[0].blocks:
        insts = bb.instructions
        i = 0
        while i < len(insts):
            inst = insts[i]
            si = inst.sync_info
            waits = list(si.on_wait) if si and si.on_wait else []
            if len(waits) > _MAX_WAITS:
                keep = waits[-_MAX_WAITS:]
                extra = waits[: -_MAX_WAITS]
                si.on_wait.clear()
                for w in keep:
                    si.on_wait.append(w)
                nops = []
                while extra:
                    chunk, extra = extra[:_MAX_WAITS], extra[_MAX_WAITS:]
                    nop = mybir.InstNoOp(name=f"I-waitnop-{n}", ins=[], outs=[])
                    n += 1
                    nop.engine = inst.engine
                    nop.sync_info = mybir.SyncInfo(on_wait=chunk, on_update=[])
                    nops.append(nop)
                insts[i:i] = nops
                i += len(nops)
            i += 1


def _fix_unsupported_isa(nc):
    """This walrus build rejects EVENT_SEMAPHORE_RANGE_CLEAR ('ISA wrong
    length'); replace it with per-semaphore write-0 EventSemaphore ops."""
    for bb in nc.m.functions[0].blocks:
        insts = bb.instructions
        idx = 0
        while idx < len(insts):
            i = insts[idx]
            if (
                type(i).__name__ == "InstISA"
                and i.op_name == "EVENT_SEMAPHORE_RANGE_CLEAR"
            ):
                d = i.ant_dict
                waits = (
                    list(i.sync_info.on_wait)
                    if i.sync_info and i.sync_info.on_wait
                    else []
                )
                repl = []
                for s in range(d["range_first"], d["range_last"] + 1):
                    ev = mybir.InstEventSemaphore(
                        name=f"I-semclr-{bb.name}-{s}", ins=[], outs=[]
                    )
                    ev.engine = i.engine
                    ev.sync_info = mybir.SyncInfo(
                        on_wait=waits if s == d["range_first"] else [],
                        on_update=[
                            mybir.SyncUpdate(
                                sync_type="semaphore",
                                id=s,
                                ant_name=f"clr{s}",
                                update_mode="sem-wr-imm",
                                update_value=0,
                                update_reg=None,
                            )
                        ],
                    )
                    repl.append(ev)
                insts[idx : idx + 1] = repl
                idx += len(repl)
            else:
                idx += 1


# packed weight layout (cols within WALL [128, 1544]):
#   wqt: 2 chunks of 256 at 0, 256
#   wkt: 2 chunks of 256 at 512, 768
#   wvt_aug: 2 chunks of 260 at 1024, 1284
_WQ0, _WQ1, _WK0, _WK1, _WV0, _WV1 = 0, 256, 512, 768, 1024, 1284
_WALL_W = 1544


def build_module(for_hw=True, repeat=1, has_bqk=False, has_bv=False, has_bo=False):
    nc = bass.Bass()

    x_d = nc.dram_tensor("x", [C, N], F32, kind="ExternalInput")
    wall_d = nc.dram_tensor("wall", [128, _WALL_W], F32, kind="ExternalInput")
    wo2_d = nc.dram_tensor("wo2", [HD, NH * C], F32, kind="ExternalInput")
    # misc: col 0 = gamma, cols 1..512 = ones
    misc_d = nc.dram_tensor("misc", [128, 513], F32, kind="ExternalInput")
    v1ones_d = nc.dram_tensor("v1ones", [128, NMC * NH], F32, kind="ExternalInput")
    bqk_d = nc.dram_tensor("bqk", [C, 2], F32, kind="ExternalInput")
    brow_d = nc.dram_tensor("brow", [1, HD1 * NH + C], F32, kind="ExternalInput")
    y_d = nc.dram_tensor("y", [C, N], F32, kind="ExternalOutput")

    with tile.TileContext(nc) as tc:
        consts = tc.alloc_tile_pool(name="consts", bufs=1)

        def ctile(shape, dtype, nm):
            return consts.tile(shape, dtype, tag=nm, name=nm)

        # ---- persistent SBUF tensors (f32r = PE matmul operands) ----
        X = [ctile([128, N], F32R, f"x{t}") for t in range(2)]
        Q = [ctile([128, N], F32R, f"q{t}") for t in range(2)]
        K = [ctile([128, N], F32R, f"k{t}") for t in range(2)]
        V1T = ctile([128, NMC * NH * HD1], F32R, "v1t")  # [mc, h, 65] in cols
        OH = [ctile([128, N], F32R, f"oh{h}") for h in range(NH)]
        WALL = ctile([128, _WALL_W], F32R, "wall")
        WO2 = ctile([128, NH * C], F32R, "wo2")
        MISC = ctile([128, 513], F32R, "misc")
        BQK = ctile([128, 4], F32, "bqk")  # [bq0|bk0] rows0-127, [bq1|bk1]
        BROW = ctile([128, HD1 * NH + C], F32R, "brow")

        GAMMA = MISC[:, 0:1].bitcast(F32)
        ONES = MISC[:, 1:513]

        # ---- loads ----
        for t in range(2):
            sl = slice(t * 128, (t + 1) * 128)
            nc.sync.dma_start(out=X[t], in_=x_d[sl, :].bitcast(F32R))
        nc.sync.dma_start(out=WALL, in_=wall_d[:, :].bitcast(F32R))
        nc.sync.dma_start(out=WO2[0:HD, :], in_=wo2_d[:, :].bitcast(F32R))
        nc.sync.dma_start(out=MISC, in_=misc_d[:, :].bitcast(F32R))
        # per-head ones columns of V1T, all via one strided DMA
        v1view = V1T.rearrange("p (m h c) -> p m h c", h=NH, c=HD1)
        nc.sync.dma_start(
            out=v1view[:, :, :, HD : HD + 1],
            in_=v1ones_d[:, :].bitcast(F32R).rearrange("p (m h) -> p m h", h=NH).unsqueeze(3),
        )
        if has_bqk:
            for t in range(2):
                nc.sync.dma_start(
                    out=BQK[:, 2 * t : 2 * t + 2], in_=bqk_d[t * 128 : (t + 1) * 128, :]
                )
        if has_bv or has_bo:
            nc.sync.dma_start(out=BROW[0:1, :], in_=brow_d[:, :].bitcast(F32R))

        for _rep in range(repeat):
            psp = tc.alloc_tile_pool(name="psp", bufs=2, space="PSUM")

            # ---- stage 1: Q, K projections ([o, n] layout), paired evac ----
            for pi, (w0, w1, DST) in enumerate(
                ((_WQ0, _WQ1, Q), (_WK0, _WK1, K))
            ):
                for ot in range(2):
                    for blo in (0, 2, 4):
                        pair = BLOCKS[blo : blo + 2]
                        ps = psp.tile([128, 1024], F32, tag="psp", name="psp")
                        for j, (n0, nw) in enumerate(pair):
                            nc.tensor.matmul(
                                ps[:, j * 512 : j * 512 + nw],
                                WALL[:, w0 + ot * 128 : w0 + ot * 128 + 128],
                                X[0][:, n0 : n0 + nw],
                                start=True,
                                stop=False,
                            )
                            nc.tensor.matmul(
                                ps[:, j * 512 : j * 512 + nw],
                                WALL[:, w1 + ot * 128 : w1 + ot * 128 + 128],
                                X[1][:, n0 : n0 + nw],
                                start=False,
                                stop=True,
                            )
                        n0, tot = pair[0][0], sum(nw for _, nw in pair)
                        src = ps[:, 0:tot]
                        dst = DST[ot][:, n0 : n0 + tot]
                        if has_bqk:
                            nc.vector.tensor_scalar_add(
                                dst, src, BQK[:, 2 * ot + pi : 2 * ot + pi + 1]
                            )
                        else:
                            nc.vector.tensor_copy(dst, src)

            # ---- stage 2: V^T (per-head ones columns pre-DMA'd) ----
            for i in range(NMC):
                ps = psp.tile([128, NH * HD1], F32, tag="psv", name="psv")
                nc.tensor.matmul(
                    ps,
                    X[0][:, i * 128 : (i + 1) * 128],
                    WALL[:, _WV0 : _WV0 + NH * HD1],
                    start=True,
                    stop=False,
                )
                nc.tensor.matmul(
                    ps,
                    X[1][:, i * 128 : (i + 1) * 128],
                    WALL[:, _WV1 : _WV1 + NH * HD1],
                    start=False,
                    stop=not has_bv,
                )
                if has_bv:
                    nc.tensor.matmul(
                        ps,
                        ONES[0:1, 0:128],
                        BROW[0:1, 0 : NH * HD1],
                        start=False,
                        stop=True,
                    )
                # copy data cols only (ones cols already set)
                nc.vector.tensor_copy(
                    v1view[:, i, :, 0:HD],
                    ps.rearrange("p (h c) -> p h c", c=HD1)[:, :, 0:HD],
                )

            # ---- stages 3+4: attention + output projection, per n-block ----
            psp.release()
            et_pool = tc.alloc_tile_pool(name="et", bufs=7)
            rc_pool = tc.alloc_tile_pool(name="rc", bufs=2)
            bc_pool = tc.alloc_tile_pool(name="bc", bufs=2)
            out_pool = tc.alloc_tile_pool(name="out", bufs=3)
            ps_s = tc.alloc_tile_pool(name="pss", bufs=1, space="PSUM")
            ps_av = tc.alloc_tile_pool(name="psav", bufs=2, space="PSUM")
            ps_bc = tc.alloc_tile_pool(name="psbc", bufs=1, space="PSUM")
            ps_o = tc.alloc_tile_pool(name="pso", bufs=1, space="PSUM")

            for n0, nw in BLOCKS:
                for h in range(NH):
                    ht, hp = divmod(h, 2)
                    qh = Q[ht][hp * HD : (hp + 1) * HD, n0 : n0 + nw]
                    ET = []  # (et_tile, col_offset) per m-chunk
                    mc0 = 0
                    for g in GRPS:
                        ps = ps_s.tile([128, GRP * 512], F32, tag="s", name="s")
                        et = et_pool.tile([128, GRP * 512], F32R, tag="et", name="et")
                        for j in range(g):
                            mc = mc0 + j
                            kh = K[ht][
                                hp * HD : (hp + 1) * HD, mc * 128 : (mc + 1) * 128
                            ]
                            nc.tensor.matmul(
                                ps[:, j * 512 : j * 512 + nw],
                                kh,
                                qh,
                                start=True,
                                stop=True,
                            )
                            ET.append((et, j * 512))
                        if nw == 512:
                            nc.scalar.activation(
                                et[:, 0 : g * 512], ps[:, 0 : g * 512], AF.Exp, scale=0.125
                            )
                        else:
                            psv_ = ps.rearrange("p (g c) -> p g c", c=512)[:, 0:g, 0:nw]
                            etv_ = et.rearrange("p (g c) -> p g c", c=512)[:, 0:g, 0:nw]
                            nc.scalar.activation(etv_, psv_, AF.Exp, scale=0.125)
                        mc0 += g
                    psa = ps_av.tile([HD1, 512], F32, tag="av", name="av")
                    for mc in range(NMC):
                        et, off = ET[mc]
                        nc.tensor.matmul(
                            psa[:, :nw],
                            v1view[:, mc, h, :],
                            et[:, off : off + nw],
                            start=(mc == 0),
                            stop=(mc == NMC - 1),
                        )
                    # normalize: sums row -> SBUF, K=1 broadcast, fused recip
                    rc = rc_pool.tile([128, 512], F32R, tag="rc", name="rc")
                    nc.vector.tensor_copy(rc[HD : HD + 1, :nw], psa[HD : HD + 1, :nw])
                    psb = ps_bc.tile([HD, 512], F32, tag="bc", name="bc")
                    nc.tensor.matmul(
                        psb[:, :nw],
                        ONES[HD : HD + 1, 0:HD],
                        rc[HD : HD + 1, :nw],
                        start=True,
                        stop=True,
                    )
                    bc = bc_pool.tile([HD, 512], F32, tag="bcs", name="bcs")
                    nc.vector.reciprocal(bc[:, :nw], psb[:, :nw])
                    nc.vector.tensor_mul(
                        OH[h][0:HD, n0 : n0 + nw], psa[0:HD, :nw], bc[:, :nw]
                    )
                # ---- output projection for this n-block + fused residual ----
                for ot in range(2):
                    pso = ps_o.tile([128, 512], F32, tag="o", name="o")
                    for h in range(NH):
                        nc.tensor.matmul(
                            pso[:, :nw],
                            WO2[0:HD, h * C + ot * 128 : h * C + ot * 128 + 128],
                            OH[h][0:HD, n0 : n0 + nw],
                            start=(h == 0),
                            stop=(h == NH - 1) and not has_bo,
                        )
                    if has_bo:
                        nc.tensor.matmul(
                            pso[:, :nw],
                            BROW[0:1, NH * HD1 + ot * 128 : NH * HD1 + ot * 128 + 128],
                            ONES[0:1, 0:nw],
                            start=False,
                            stop=True,
                        )
                    outt = out_pool.tile([128, 512], F32, tag="out", name="out")
                    nc.vector.scalar_tensor_tensor(
                        outt[:, :nw],
                        pso[:, :nw],
                        GAMMA,
                        X[ot][:, n0 : n0 + nw].bitcast(F32),
                        op0=mybir.AluOpType.mult,
                        op1=mybir.AluOpType.add,
                    )
                    nc.sync.dma_start(
                        out=y_d[ot * 128 : (ot + 1) * 128, n0 : n0 + nw],
                        in_=outt[:, :nw],
                    )

            for p in (ps_o, ps_bc, ps_av, ps_s, out_pool, bc_pool, rc_pool, et_pool):
                p.release()
        consts.release()

    if for_hw:
        # walrus-compat rewrites; CoreSim can't execute post-hoc instructions
        _fix_unsupported_isa(nc)
        _split_multi_waits(nc)
    return nc


def make_in_maps(x, Wq, bq, Wk, bk, Wv, bv, Wo, bo, gamma):
    x = np.asarray(x, dtype=np.float32)
    B = x.shape[0]
    gamma = np.asarray(gamma, dtype=np.float32).reshape(-1)[0]
    f = lambda a: np.asarray(a, np.float32)

    wall = np.zeros((128, _WALL_W), np.float32)
    wqt, wkt, wvt = f(Wq).T, f(Wk).T, f(Wv).T
    wall[:, _WQ0 : _WQ0 + 256] = wqt[0:128]
    wall[:, _WQ1 : _WQ1 + 256] = wqt[128:256]
    wall[:, _WK0 : _WK0 + 256] = wkt[0:128]
    wall[:, _WK1 : _WK1 + 256] = wkt[128:256]
    wvt_aug = np.zeros((C, NH * HD1), np.float32)
    for h in range(NH):
        wvt_aug[:, h * HD1 : h * HD1 + HD] = wvt[:, h * HD : (h + 1) * HD]
    wall[:, _WV0 : _WV0 + NH * HD1] = wvt_aug[0:128]
    wall[:, _WV1 : _WV1 + NH * HD1] = wvt_aug[128:256]

    wot = f(Wo).T  # [c, o]
    wo2 = np.zeros((HD, NH * C), np.float32)
    for h in range(NH):
        wo2[:, h * C : (h + 1) * C] = wot[h * HD : (h + 1) * HD, :]

    misc = np.ones((128, 513), np.float32)
    misc[:, 0] = gamma

    bv_arr, bo_arr = f(bv).reshape(C), f(bo).reshape(C)
    brow = np.zeros((1, HD1 * NH + C), np.float32)
    for h in range(NH):
        brow[0, h * HD1 : h * HD1 + HD] = bv_arr[h * HD : (h + 1) * HD]
    brow[0, NH * HD1 :] = bo_arr

    bqk = np.stack([f(bq).reshape(C), f(bk).reshape(C)], axis=1)

    common = {
        "wall": wall,
        "wo2": wo2,
        "misc": misc,
        "v1ones": np.ones((128, NMC * NH), np.float32),
        "bqk": bqk,
        "brow": brow,
    }
    flags = {
        "has_bqk": bool(np.any(bqk)),
        "has_bv": bool(np.any(bv_arr)),
        "has_bo": bool(np.any(bo_arr)),
    }
    in_maps = [
        {"x": np.ascontiguousarray(x[b].reshape(C, -1)), **common} for b in range(B)
    ]
    return in_maps, flags


_NC_CACHE = {}


def kernel(x, Wq, bq, Wk, bk, Wv, bv, Wo, bo, gamma):
    from concourse.bass_utils import run_bass_kernel_spmd

    x = np.asarray(x)
    B, Cc, H, W = x.shape
    in_maps, flags = make_in_maps(x, Wq, bq, Wk, bk, Wv, bv, Wo, bo, gamma)
    key = tuple(sorted(flags.items()))
    if key not in _NC_CACHE:
        _NC_CACHE[key] = build_module(**flags)
    res = run_bass_kernel_spmd(
        _NC_CACHE[key], in_maps, core_ids=list(range(len(in_maps)))
    )
    y = np.stack([res.results[b]["y"].reshape(Cc, H, W) for b in range(B)])
    return y.astype(x.dtype)



# revision 14
# speedup vs baseline: 1.3243x; 1.3243x over previous
"""Trainium2 Bass kernel for CrossScaleAttention.

Computes, for input x [B=8, C=256, H=48, W=48] (N = H*W = 2304):
    q = Wq x + bq ; k = Wk x + bk ; v = Wv x + bv       (1x1 conv projections)
    per head h (4 heads, d=64): attn = softmax(q_h^T k_h / 8)
    o_h = v_h attn^T ; out = Wo o + bo ; y = x + gamma * out

Sharding: data-parallel over batch; core b handles batch element b.
No collectives; each core loads its slice + replicated weights and
writes its output slice.

Device algorithm per core (all matmuls fp32r, every output >= 256 cols
so 1 col/cycle):
  - Q, K in native [o, n] layout:  Q = WqT^T @ X   (lhsT = Wq^T chunks)
  - V^T directly via  V1T = X^T @ WvT  (lhsT = X chunks) with a ones
    column per head so the attention A@V matmul also produces the
    softmax row-sums for free.
  - Scores computed TRANSPOSED (S^T[m, n] = k^T q) so no PE transposes
    are needed anywhere.
  - exp(S^T/8) is SPLIT between two engines so it overlaps the PE:
    4 of every 6 chunk-groups evacuate through the scalar engine
    (fused exp on the PSUM->SBUF copy), the other 2 through the vector
    engine using a Schraudolph integer exp emitted as int16 writes of
    bf16 bit patterns (max rel err ~3%, which the softmax
    normalization cancels to ~4e-4 end to end). E and V are bf16 (the
    PE requires matched operand widths).
  - n is processed in 6 blocks of 384 columns; score PSUM tiles hold 3
    m-chunks in 512-aligned slots (3 banks) and are double-buffered so
    the PE streams ahead while ACT/DVE drain previous groups.
  - AV: psum[65, 384] += V1T_chunk[128, 65]^T @ E^T_chunk[128, 384]
    over 18 m-chunks; row 64 = softmax denominators.
  - normalize: sums row copied to SBUF, K=1 PE broadcast matmul into
    the PSUM bank shared with the O projection, DVE reciprocal, DVE
    multiply while evacuating to OH.
  - O projection with K=64 per-head chunks of Wo^T; residual fused on
    DVE: y = proj * gamma + x  (+ bias terms when nonzero).

All f32 tiles that feed PE matmuls are declared float32r (the BIR
verifier requires fp32r operands to be produced as fp32r); DMA loads
bitcast the f32 DRAM side, and compute producers write with fp32r
output dtype. The attention AV matmul runs on bf16 operands.
"""

import math

import numpy as np

import concourse.bass as bass
import concourse.mybir as mybir
import concourse.tile as tile

F32 = mybir.dt.float32
F32R = mybir.dt.float32r
BF16 = mybir.dt.bfloat16
I16 = mybir.dt.int16
AF = mybir.ActivationFunctionType
ALU = mybir.AluOpType

C = 256
N = 2304  # 48*48
NH = 4
HD = 64  # head dim
HD1 = HD + 1
KC = 128  # contraction chunk
NMC = N // KC  # 18 m-chunks
NW = 384
BLOCKS = [(i * NW, NW) for i in range(N // NW)]  # 6 blocks
NG = 6  # chunk-groups per (block, head); 3 m-chunks each in 512-col slots
ASSIGN = "ADAADA"  # exp engine per group: A=scalar(exp), D=vector(int exp)

# Schraudolph integer exp constants for exp(s * 0.125), emitted as int16
# writes of bf16 bit patterns (bf16 shares f32's 8-bit exponent, so the
# classic trick works at 1/2^16 scale; c = 366393/2^16 ~ 5.59)
_SCH_A = 0.125 * (1 << 7) / math.log(2.0)
_SCH_B = float(127 << 7) - 366393.0 / (1 << 16) + 0.5

_MAX_WAITS = 1  # walrus in this environment accepts 1 sync-wait per instruction


def _split_multi_waits(nc):
    """Hoist excess sem-waits onto same-engine NoOps emitted just before the
    owning instruction (the engine stalls at the NoOp instead — identical
    semantics, one wait per instruction)."""
    n = 0
    for bb in nc.m.functions[0].blocks:
        insts = bb.instructions
        i = 0
        while i < len(insts):
            inst = insts[i]
            si = inst.sync_info
            waits = list(si.on_wait) if si and si.on_wait else []
            if len(waits) > _MAX_WAITS:
                keep = waits[-_MAX_WAITS:]
                extra = waits[: -_MAX_WAITS]
                si.on_wait.clear()
                for w in keep:
                    si.on_wait.append(w)
                nops = []
                while extra:
                    chunk, extra = extra[:_MAX_WAITS], extra[_MAX_WAITS:]
                    nop = mybir.InstNoOp(name=f"I-waitnop-{n}", ins=[], outs=[])
                    n += 1
                    nop.engine = inst.engine
                    nop.sync_info = mybir.SyncInfo(on_wait=chunk, on_update=[])
                    nops.append(nop)
                insts[i:i] = nops
                i += len(nops)
            i += 1


def _fix_unsupported_isa(nc):
    """This walrus build rejects EVENT_SEMAPHORE_RANGE_CLEAR ('ISA wrong
    length'); replace it with per-semaphore write-0 EventSemaphore ops."""
    for bb in nc.m.functions[0].blocks:
        insts = bb.instructions
        idx = 0
        while idx < len(insts):
            i = insts[idx]
            if (
                type(i).__name__ == "InstISA"
                and i.op_name == "EVENT_SEMAPHORE_RANGE_CLEAR"
            ):
                d = i.ant_dict
                waits = (
                    list(i.sync_info.on_wait)
                    if i.sync_info and i.sync_info.on_wait
                    else []
                )
                repl = []
                for s in range(d["range_first"], d["range_last"] + 1):
                    ev = mybir.InstEventSemaphore(
                        name=f"I-semclr-{bb.name}-{s}", ins=[], outs=[]
                    )
                    ev.engine = i.engine
                    ev.sync_info = mybir.SyncInfo(
                        on_wait=waits if s == d["range_first"] else [],
                        on_update=[
                            mybir.SyncUpdate(
                                sync_type="semaphore",
                                id=s,
                                ant_name=f"clr{s}",
                                update_mode="sem-wr-imm",
                                update_value=0,
                                update_reg=None,
                            )
                        ],
                    )
                    repl.append(ev)
                insts[idx : idx + 1] = repl
                idx += len(repl)
            else:
                idx += 1


# packed weight layout (cols within WALL [128, 1544]):
#   wqt: 2 chunks of 256 at 0, 256
#   wkt: 2 chunks of 256 at 512, 768
#   wvt_aug: 2 chunks of 260 at 1024, 1284
_WQ0, _WQ1, _WK0, _WK1, _WV0, _WV1 = 0, 256, 512, 768, 1024, 1284
_WALL_W = 1544


def build_module(for_hw=True, repeat=1, has_bqk=False, has_bv=False, has_bo=False):
    nc = bass.Bass()

    x_d = nc.dram_tensor("x", [C, N], F32, kind="ExternalInput")
    wall_d = nc.dram_tensor("wall", [128, _WALL_W], F32, kind="ExternalInput")
    wo2_d = nc.dram_tensor("wo2", [HD, NH * C], F32, kind="ExternalInput")
    # misc: col 0 = gamma, cols 1..512 = ones
    misc_d = nc.dram_tensor("misc", [128, 513], F32, kind="ExternalInput")
    v1ones_d = nc.dram_tensor("v1ones", [128, NMC * NH], BF16, kind="ExternalInput")
    bqk_d = nc.dram_tensor("bqk", [C, 2], F32, kind="ExternalInput")
    brow_d = nc.dram_tensor("brow", [1, HD1 * NH + C], F32, kind="ExternalInput")
    y_d = nc.dram_tensor("y", [C, N], F32, kind="ExternalOutput")

    with tile.TileContext(nc) as tc:
        consts = tc.alloc_tile_pool(name="consts", bufs=1)

        def ctile(shape, dtype, nm):
            return consts.tile(shape, dtype, tag=nm, name=nm)

        # ---- persistent SBUF tensors (f32r = PE matmul operands) ----
        X = [ctile([128, N], F32R, f"x{t}") for t in range(2)]
        Q = [ctile([128, N], F32R, f"q{t}") for t in range(2)]
        K = [ctile([128, N], F32R, f"k{t}") for t in range(2)]
        V1T = ctile([128, NMC * NH * HD1], BF16, "v1t")  # [mc, h, 65] in cols
        OH = [ctile([128, N], F32R, f"oh{h}") for h in range(NH)]
        WALL = ctile([128, _WALL_W], F32R, "wall")
        WO2 = ctile([128, NH * C], F32R, "wo2")
        MISC = ctile([128, 513], F32R, "misc")
        BQK = ctile([128, 4], F32, "bqk")  # [bq0|bk0] rows0-127, [bq1|bk1]
        BROW = ctile([128, HD1 * NH + C], F32R, "brow")

        GAMMA = MISC[:, 0:1].bitcast(F32)
        ONES = MISC[:, 1:513]

        # ---- loads ----
        for t in range(2):
            sl = slice(t * 128, (t + 1) * 128)
            nc.sync.dma_start(out=X[t], in_=x_d[sl, :].bitcast(F32R))
        nc.sync.dma_start(out=WALL, in_=wall_d[:, :].bitcast(F32R))
        nc.sync.dma_start(out=WO2[0:HD, :], in_=wo2_d[:, :].bitcast(F32R))
        nc.sync.dma_start(out=MISC, in_=misc_d[:, :].bitcast(F32R))
        # per-head ones columns of V1T, all via one strided DMA
        v1view = V1T.rearrange("p (m h c) -> p m h c", h=NH, c=HD1)
        nc.sync.dma_start(
            out=v1view[:, :, :, HD : HD + 1],
            in_=v1ones_d[:, :].rearrange("p (m h) -> p m h", h=NH).unsqueeze(3),
        )
        if has_bqk:
            for t in range(2):
                nc.sync.dma_start(
                    out=BQK[:, 2 * t : 2 * t + 2], in_=bqk_d[t * 128 : (t + 1) * 128, :]
                )
        if has_bv or has_bo:
            nc.sync.dma_start(out=BROW[0:1, :], in_=brow_d[:, :].bitcast(F32R))

        for _rep in range(repeat):
            psp = tc.alloc_tile_pool(name="psp", bufs=2, space="PSUM")

            # ---- stage 1: Q, K projections ([o, n] layout), paired evac ----
            for pi, (w0, w1, DST) in enumerate(((_WQ0, _WQ1, Q), (_WK0, _WK1, K))):
                for ot in range(2):
                    for blo in range(3):
                        n0 = blo * 768
                        nw2 = min(1024, N - blo * 1024)
                        ps = psp.tile([128, 1024], F32, tag="psp", name="psp")
                        for j, jn0 in enumerate((blo * 1024, blo * 1024 + 512)):
                            if jn0 >= N:
                                continue
                            jw = min(512, N - jn0)
                            nc.tensor.matmul(
                                ps[:, j * 512 : j * 512 + jw],
                                WALL[:, w0 + ot * 128 : w0 + ot * 128 + 128],
                                X[0][:, jn0 : jn0 + jw],
                                start=True,
                                stop=False,
                            )
                            nc.tensor.matmul(
                                ps[:, j * 512 : j * 512 + jw],
                                WALL[:, w1 + ot * 128 : w1 + ot * 128 + 128],
                                X[1][:, jn0 : jn0 + jw],
                                start=False,
                                stop=True,
                            )
                        tot = min(1024, N - blo * 1024)
                        src = ps[:, 0:tot]
                        dst = DST[ot][:, blo * 1024 : blo * 1024 + tot]
                        if has_bqk:
                            nc.vector.tensor_scalar_add(
                                dst, src, BQK[:, 2 * ot + pi : 2 * ot + pi + 1]
                            )
                        else:
                            nc.vector.tensor_copy(dst, src)

            # ---- stage 2: V^T (per-head ones columns pre-DMA'd) ----
            for i in range(NMC):
                ps = psp.tile([128, NH * HD1], F32, tag="psv", name="psv")
                nc.tensor.matmul(
                    ps,
                    X[0][:, i * 128 : (i + 1) * 128],
                    WALL[:, _WV0 : _WV0 + NH * HD1],
                    start=True,
                    stop=False,
                )
                nc.tensor.matmul(
                    ps,
                    X[1][:, i * 128 : (i + 1) * 128],
                    WALL[:, _WV1 : _WV1 + NH * HD1],
                    start=False,
                    stop=not has_bv,
                )
                if has_bv:
                    nc.tensor.matmul(
                        ps,
                        ONES[0:1, 0:128],
                        BROW[0:1, 0 : NH * HD1],
                        start=False,
                        stop=True,
                    )
                # copy data cols only (ones cols already set)
                nc.vector.tensor_copy(
                    v1view[:, i, :, 0:HD],
                    ps.rearrange("p (h c) -> p h c", c=HD1)[:, :, 0:HD],
                )

            # ---- stages 3+4: attention + output projection, per n-block ----
            psp.release()
            et_pool = tc.alloc_tile_pool(name="et", bufs=7)
            rc_pool = tc.alloc_tile_pool(name="rc", bufs=2)
            bc_pool = tc.alloc_tile_pool(name="bc", bufs=2)
            out_pool = tc.alloc_tile_pool(name="out", bufs=3)
            ps_s = tc.alloc_tile_pool(name="pss", bufs=2, space="PSUM")
            ps_av = tc.alloc_tile_pool(name="psav", bufs=1, space="PSUM")
            ps_o = tc.alloc_tile_pool(name="pso", bufs=1, space="PSUM")

            for n0, nw in BLOCKS:
                for h in range(NH):
                    ht, hp = divmod(h, 2)
                    qh = Q[ht][hp * HD : (hp + 1) * HD, n0 : n0 + nw]
                    ET = []  # (et_tile, col_offset) per m-chunk
                    for g in range(NG):
                        # 3 m-chunks per group, 512-aligned slots (bank-safe)
                        ps = ps_s.tile([128, 1536], F32, tag="s", name="s")
                        et = et_pool.tile([128, 1536], BF16, tag="et", name="et")
                        for j in range(3):
                            mc = g * 3 + j
                            kh = K[ht][
                                hp * HD : (hp + 1) * HD, mc * 128 : (mc + 1) * 128
                            ]
                            nc.tensor.matmul(
                                ps[:, j * 512 : j * 512 + nw],
                                kh,
                                qh,
                                start=True,
                                stop=True,
                            )
                            ET.append((et, j * 512))
                        psv_ = ps.rearrange("p (g c) -> p g c", c=512)[:, 0:3, 0:nw]
                        if ASSIGN[g] == "A":
                            etv_ = et.rearrange("p (g c) -> p g c", c=512)[:, 0:3, 0:nw]
                            nc.scalar.activation(etv_, psv_, AF.Exp, scale=0.125)
                        else:
                            eti_ = et.bitcast(I16).rearrange(
                                "p (g c) -> p g c", c=512
                            )[:, 0:3, 0:nw]
                            nc.vector.tensor_scalar(
                                eti_, psv_, _SCH_A, _SCH_B, ALU.mult, ALU.add
                            )
                    psa = ps_av.tile([HD1, NW], F32, tag="av", name="av")
                    for mc in range(NMC):
                        et, off = ET[mc]
                        nc.tensor.matmul(
                            psa[:, :nw],
                            v1view[:, mc, h, :],
                            et[:, off : off + nw],
                            start=(mc == 0),
                            stop=(mc == NMC - 1),
                        )
                    # normalize: sums row -> SBUF, K=1 PE broadcast into the
                    # shared o/bc PSUM bank, reciprocal, multiply on the
                    # PSUM->SBUF evacuation.
                    rc = rc_pool.tile([128, NW], F32R, tag="rc", name="rc")
                    nc.vector.tensor_copy(rc[HD : HD + 1, :nw], psa[HD : HD + 1, :nw])
                    psb = ps_o.tile([128, NW], F32, tag="o", name="o")
                    nc.tensor.matmul(
                        psb[0:HD, :nw],
                        ONES[HD : HD + 1, 0:HD],
                        rc[HD : HD + 1, :nw],
                        start=True,
                        stop=True,
                    )
                    bc = bc_pool.tile([HD, NW], F32, tag="bcs", name="bcs")
                    nc.vector.reciprocal(bc[:, :nw], psb[0:HD, :nw])
                    nc.vector.tensor_mul(
                        OH[h][0:HD, n0 : n0 + nw], psa[0:HD, :nw], bc[:, :nw]
                    )
                # ---- output projection for this n-block + fused residual ----
                for ot in range(2):
                    pso = ps_o.tile([128, NW], F32, tag="o", name="o")
                    for h in range(NH):
                        nc.tensor.matmul(
                            pso[:, :nw],
                            WO2[0:HD, h * C + ot * 128 : h * C + ot * 128 + 128],
                            OH[h][0:HD, n0 : n0 + nw],
                            start=(h == 0),
                            stop=(h == NH - 1) and not has_bo,
                        )
                    if has_bo:
                        nc.tensor.matmul(
                            pso[:, :nw],
                            BROW[0:1, NH * HD1 + ot * 128 : NH * HD1 + ot * 128 + 128],
                            ONES[0:1, 0:nw],
                            start=False,
                            stop=True,
                        )
                    outt = out_pool.tile([128, NW], F32, tag="out", name="out")
                    nc.vector.scalar_tensor_tensor(
                        outt[:, :nw],
                        pso[:, :nw],
                        GAMMA,
                        X[ot][:, n0 : n0 + nw].bitcast(F32),
                        op0=mybir.AluOpType.mult,
                        op1=mybir.AluOpType.add,
                    )
                    nc.sync.dma_start(
                        out=y_d[ot * 128 : (ot + 1) * 128, n0 : n0 + nw],
                        in_=outt[:, :nw],
                    )

            for p in (ps_o, ps_av, ps_s, out_pool, bc_pool, rc_pool, et_pool):
                p.release()
        consts.release()

    if for_hw:
        # walrus-compat rewrites; CoreSim can't execute post-hoc instructions
        _fix_unsupported_isa(nc)
        _split_multi_waits(nc)
    return nc


def make_in_maps(x, Wq, bq, Wk, bk, Wv, bv, Wo, bo, gamma):
    x = np.asarray(x, dtype=np.float32)
    B = x.shape[0]
    gamma = np.asarray(gamma, dtype=np.float32).reshape(-1)[0]
    f = lambda a: np.asarray(a, np.float32)

    wall = np.zeros((128, _WALL_W), np.float32)
    wqt, wkt, wvt = f(Wq).T, f(Wk).T, f(Wv).T
    wall[:, _WQ0 : _WQ0 + 256] = wqt[0:128]
    wall[:, _WQ1 : _WQ1 + 256] = wqt[128:256]
    wall[:, _WK0 : _WK0 + 256] = wkt[0:128]
    wall[:, _WK1 : _WK1 + 256] = wkt[128:256]
    wvt_aug = np.zeros((C, NH * HD1), np.float32)
    for h in range(NH):
        wvt_aug[:, h * HD1 : h * HD1 + HD] = wvt[:, h * HD : (h + 1) * HD]
    wall[:, _WV0 : _WV0 + NH * HD1] = wvt_aug[0:128]
    wall[:, _WV1 : _WV1 + NH * HD1] = wvt_aug[128:256]

    wot = f(Wo).T  # [c, o]
    wo2 = np.zeros((HD, NH * C), np.float32)
    for h in range(NH):
        wo2[:, h * C : (h + 1) * C] = wot[h * HD : (h + 1) * HD, :]

    misc = np.ones((128, 513), np.float32)
    misc[:, 0] = gamma

    bv_arr, bo_arr = f(bv).reshape(C), f(bo).reshape(C)
    brow = np.zeros((1, HD1 * NH + C), np.float32)
    for h in range(NH):
        brow[0, h * HD1 : h * HD1 + HD] = bv_arr[h * HD : (h + 1) * HD]
    brow[0, NH * HD1 :] = bo_arr

    bqk = np.stack([f(bq).reshape(C), f(bk).reshape(C)], axis=1)

    common = {
        "wall": wall,
        "wo2": wo2,
        "misc": misc,
        "v1ones": np.ones((128, NMC * NH), __import__("ml_dtypes").bfloat16),
        "bqk": bqk,
        "brow": brow,
    }
    flags = {
        "has_bqk": bool(np.any(bqk)),
        "has_bv": bool(np.any(bv_arr)),
        "has_bo": bool(np.any(bo_arr)),
    }
    in_maps = [
        {"x": np.ascontiguousarray(x[b].reshape(C, -1)), **common} for b in range(B)
    ]
    return in_maps, flags


_NC_CACHE = {}


def kernel(x, Wq, bq, Wk, bk, Wv, bv, Wo, bo, gamma):
    from concourse.bass_utils import run_bass_kernel_spmd

    x = np.asarray(x)
    B, Cc, H, W = x.shape
    in_maps, flags = make_in_maps(x, Wq, bq, Wk, bk, Wv, bv, Wo, bo, gamma)
    key = tuple(sorted(flags.items()))
    if key not in _NC_CACHE:
        _NC_CACHE[key] = build_module(**flags)
    res = run_bass_kernel_spmd(
        _NC_CACHE[key], in_maps, core_ids=list(range(len(in_maps)))
    )
    y = np.stack([res.results[b]["y"].reshape(Cc, H, W) for b in range(B)])
    return y.astype(x.dtype)


# revision 17
# speedup vs baseline: 1.3552x; 1.0233x over previous
"""Trainium2 Bass kernel for CrossScaleAttention.

Computes, for input x [B=8, C=256, H=48, W=48] (N = H*W = 2304):
    q = Wq x + bq ; k = Wk x + bk ; v = Wv x + bv       (1x1 conv projections)
    per head h (4 heads, d=64): attn = softmax(q_h^T k_h / 8)
    o_h = v_h attn^T ; out = Wo o + bo ; y = x + gamma * out

Sharding: data-parallel over batch; core b handles batch element b.
No collectives; each core loads its slice + replicated weights and
writes its output slice.

Device algorithm per core (all matmuls fp32r, every output >= 256 cols
so 1 col/cycle):
  - Q, K in native [o, n] layout:  Q = WqT^T @ X   (lhsT = Wq^T chunks)
  - V^T directly via  V1T = X^T @ WvT  (lhsT = X chunks) with a ones
    column per head so the attention A@V matmul also produces the
    softmax row-sums for free.
  - Scores computed TRANSPOSED (S^T[m, n] = k^T q) so no PE transposes
    are needed anywhere.
  - exp(S^T/8) is SPLIT between two engines so it overlaps the PE:
    4 of every 6 chunk-groups evacuate through the scalar engine
    (fused exp on the PSUM->SBUF copy), the other 2 through the vector
    engine using a Schraudolph integer exp emitted as int16 writes of
    bf16 bit patterns (max rel err ~3%, which the softmax
    normalization cancels to ~4e-4 end to end). E and V are bf16 (the
    PE requires matched operand widths).
  - n is processed in 6 blocks of 384 columns; score PSUM tiles hold 3
    m-chunks in 512-aligned slots (3 banks) and are double-buffered so
    the PE streams ahead while ACT/DVE drain previous groups.
  - AV: psum[65, 384] += V1T_chunk[128, 65]^T @ E^T_chunk[128, 384]
    over 18 m-chunks; row 64 = softmax denominators.
  - normalize: sums row copied to SBUF, K=1 PE broadcast matmul into
    the PSUM bank shared with the O projection, DVE reciprocal, DVE
    multiply while evacuating to OH.
  - O projection with K=64 per-head chunks of Wo^T; residual fused on
    DVE: y = proj * gamma + x  (+ bias terms when nonzero).

All f32 tiles that feed PE matmuls are declared float32r (the BIR
verifier requires fp32r operands to be produced as fp32r); DMA loads
bitcast the f32 DRAM side, and compute producers write with fp32r
output dtype. The attention AV matmul runs on bf16 operands.
"""

import math

import numpy as np

import concourse.bass as bass
import concourse.mybir as mybir
import concourse.tile as tile

F32 = mybir.dt.float32
F32R = mybir.dt.float32r
BF16 = mybir.dt.bfloat16
I16 = mybir.dt.int16
AF = mybir.ActivationFunctionType
ALU = mybir.AluOpType

C = 256
N = 2304  # 48*48
NH = 4
HD = 64  # head dim
HD1 = HD + 1
KC = 128  # contraction chunk
NMC = N // KC  # 18 m-chunks
NW = 384
BLOCKS = [(i * NW, NW) for i in range(N // NW)]  # 6 blocks
NG = 6  # chunk-groups per (block, head); 3 m-chunks each in 512-col slots
ASSIGN = "ADAADA"  # exp engine per group: A=scalar(exp), D=vector(int exp)

# Schraudolph integer exp constants for exp(s * 0.125), emitted as int16
# writes of bf16 bit patterns (bf16 shares f32's 8-bit exponent, so the
# classic trick works at 1/2^16 scale; c = 366393/2^16 ~ 5.59)
_SCH_A = 0.125 * (1 << 7) / math.log(2.0)
_SCH_B = float(127 << 7) - 366393.0 / (1 << 16) + 0.5

_MAX_WAITS = 1  # walrus in this environment accepts 1 sync-wait per instruction


def _split_multi_waits(nc):
    """Hoist excess sem-waits onto same-engine NoOps emitted just before the
    owning instruction (the engine stalls at the NoOp instead — identical
    semantics, one wait per instruction)."""
    n = 0
    for bb in nc.m.functions[0].blocks:
        insts = bb.instructions
        i = 0
        while i < len(insts):
            inst = insts[i]
            si = inst.sync_info
            waits = list(si.on_wait) if si and si.on_wait else []
            if len(waits) > _MAX_WAITS:
                keep = waits[-_MAX_WAITS:]
                extra = waits[: -_MAX_WAITS]
                si.on_wait.clear()
                for w in keep:
                    si.on_wait.append(w)
                nops = []
                while extra:
                    chunk, extra = extra[:_MAX_WAITS], extra[_MAX_WAITS:]
                    nop = mybir.InstNoOp(name=f"I-waitnop-{n}", ins=[], outs=[])
                    n += 1
                    nop.engine = inst.engine
                    nop.sync_info = mybir.SyncInfo(on_wait=chunk, on_update=[])
                    nops.append(nop)
                insts[i:i] = nops
                i += len(nops)
            i += 1


def _fix_unsupported_isa(nc):
    """This walrus build rejects EVENT_SEMAPHORE_RANGE_CLEAR ('ISA wrong
    length'); replace it with per-semaphore write-0 EventSemaphore ops."""
    for bb in nc.m.functions[0].blocks:
        insts = bb.instructions
        idx = 0
        while idx < len(insts):
            i = insts[idx]
            if (
                type(i).__name__ == "InstISA"
                and i.op_name == "EVENT_SEMAPHORE_RANGE_CLEAR"
            ):
                d = i.ant_dict
                waits = (
                    list(i.sync_info.on_wait)
                    if i.sync_info and i.sync_info.on_wait
                    else []
                )
                repl = []
                for s in range(d["range_first"], d["range_last"] + 1):
                    ev = mybir.InstEventSemaphore(
                        name=f"I-semclr-{bb.name}-{s}", ins=[], outs=[]
                    )
                    ev.engine = i.engine
                    ev.sync_info = mybir.SyncInfo(
                        on_wait=waits if s == d["range_first"] else [],
                        on_update=[
                            mybir.SyncUpdate(
                                sync_type="semaphore",
                                id=s,
                                ant_name=f"clr{s}",
                                update_mode="sem-wr-imm",
                                update_value=0,
                                update_reg=None,
                            )
                        ],
                    )
                    repl.append(ev)
                insts[idx : idx + 1] = repl
                idx += len(repl)
            else:
                idx += 1


# packed weight layout (cols within WALL [128, 1544]):
#   wqt: 2 chunks of 256 at 0, 256
#   wkt: 2 chunks of 256 at 512, 768
#   wvt_aug: 2 chunks of 260 at 1024, 1284
_WQ0, _WQ1, _WK0, _WK1, _WV0, _WV1 = 0, 256, 512, 768, 1024, 1284
_WALL_W = 1544


def build_module(for_hw=True, repeat=1, has_bqk=False, has_bv=False, has_bo=False):
    nc = bass.Bass()

    x_d = nc.dram_tensor("x", [C, N], F32, kind="ExternalInput")
    wall_d = nc.dram_tensor("wall", [128, _WALL_W], F32, kind="ExternalInput")
    wo2_d = nc.dram_tensor("wo2", [HD, NH * C], F32, kind="ExternalInput")
    # misc: col 0 = gamma, cols 1..512 = ones
    misc_d = nc.dram_tensor("misc", [128, 513], F32, kind="ExternalInput")
    v1ones_d = nc.dram_tensor("v1ones", [128, NMC * NH], BF16, kind="ExternalInput")
    bqk_d = nc.dram_tensor("bqk", [C, 2], F32, kind="ExternalInput")
    brow_d = nc.dram_tensor("brow", [1, HD1 * NH + C], F32, kind="ExternalInput")
    y_d = nc.dram_tensor("y", [C, N], F32, kind="ExternalOutput")

    with tile.TileContext(nc) as tc:
        consts = tc.alloc_tile_pool(name="consts", bufs=1)

        def ctile(shape, dtype, nm):
            return consts.tile(shape, dtype, tag=nm, name=nm)

        # ---- persistent SBUF tensors (f32r = PE matmul operands) ----
        X = [ctile([128, N], F32R, f"x{t}") for t in range(2)]
        Q = [ctile([128, N], F32R, f"q{t}") for t in range(2)]
        K = [ctile([128, N], F32R, f"k{t}") for t in range(2)]
        V1T = ctile([128, NMC * NH * HD1], BF16, "v1t")  # [mc, h, 65] in cols
        OH = [ctile([128, N], F32R, f"oh{h}") for h in range(NH)]
        WALL = ctile([128, _WALL_W], F32R, "wall")
        WO2 = ctile([128, NH * C], F32R, "wo2")
        MISC = ctile([128, 513], F32R, "misc")
        BQK = ctile([128, 4], F32, "bqk")  # [bq0|bk0] rows0-127, [bq1|bk1]
        BROW = ctile([128, HD1 * NH + C], F32R, "brow")

        GAMMA = MISC[:, 0:1].bitcast(F32)
        ONES = MISC[:, 1:513]

        # ---- loads ----
        nc.sync.dma_start(out=WALL, in_=wall_d[:, :].bitcast(F32R))
        for t in range(2):
            sl = slice(t * 128, (t + 1) * 128)
            nc.sync.dma_start(out=X[t], in_=x_d[sl, :].bitcast(F32R))
        nc.sync.dma_start(out=WO2[0:HD, :], in_=wo2_d[:, :].bitcast(F32R))
        nc.sync.dma_start(out=MISC, in_=misc_d[:, :].bitcast(F32R))
        # per-head ones columns of V1T, all via one strided DMA
        v1view = V1T.rearrange("p (m h c) -> p m h c", h=NH, c=HD1)
        nc.sync.dma_start(
            out=v1view[:, :, :, HD : HD + 1],
            in_=v1ones_d[:, :].rearrange("p (m h) -> p m h", h=NH).unsqueeze(3),
        )
        if has_bqk:
            for t in range(2):
                nc.sync.dma_start(
                    out=BQK[:, 2 * t : 2 * t + 2], in_=bqk_d[t * 128 : (t + 1) * 128, :]
                )
        if has_bv or has_bo:
            nc.sync.dma_start(out=BROW[0:1, :], in_=brow_d[:, :].bitcast(F32R))

        for _rep in range(repeat):
            psp = tc.alloc_tile_pool(name="psp", bufs=2, space="PSUM")

            # ---- stage 1: Q, K projections ([o, n] layout), paired evac ----
            for pi, (w0, w1, DST) in enumerate(((_WQ0, _WQ1, Q), (_WK0, _WK1, K))):
                for ot in range(2):
                    for blo in range(3):
                        n0 = blo * 768
                        nw2 = min(1024, N - blo * 1024)
                        ps = psp.tile([128, 1024], F32, tag="psp", name="psp")
                        for j, jn0 in enumerate((blo * 1024, blo * 1024 + 512)):
                            if jn0 >= N:
                                continue
                            jw = min(512, N - jn0)
                            nc.tensor.matmul(
                                ps[:, j * 512 : j * 512 + jw],
                                WALL[:, w0 + ot * 128 : w0 + ot * 128 + 128],
                                X[0][:, jn0 : jn0 + jw],
                                start=True,
                                stop=False,
                            )
                            nc.tensor.matmul(
                                ps[:, j * 512 : j * 512 + jw],
                                WALL[:, w1 + ot * 128 : w1 + ot * 128 + 128],
                                X[1][:, jn0 : jn0 + jw],
                                start=False,
                                stop=True,
                            )
                        tot = min(1024, N - blo * 1024)
                        src = ps[:, 0:tot]
                        dst = DST[ot][:, blo * 1024 : blo * 1024 + tot]
                        if has_bqk:
                            nc.vector.tensor_scalar_add(
                                dst, src, BQK[:, 2 * ot + pi : 2 * ot + pi + 1]
                            )
                        else:
                            nc.scalar.copy(dst, src)

            # ---- stage 2: V^T (per-head ones columns pre-DMA'd) ----
            for i in range(NMC):
                ps = psp.tile([128, NH * HD1], F32, tag="psv", name="psv")
                nc.tensor.matmul(
                    ps,
                    X[0][:, i * 128 : (i + 1) * 128],
                    WALL[:, _WV0 : _WV0 + NH * HD1],
                    start=True,
                    stop=False,
                )
                nc.tensor.matmul(
                    ps,
                    X[1][:, i * 128 : (i + 1) * 128],
                    WALL[:, _WV1 : _WV1 + NH * HD1],
                    start=False,
                    stop=not has_bv,
                )
                if has_bv:
                    nc.tensor.matmul(
                        ps,
                        ONES[0:1, 0:128],
                        BROW[0:1, 0 : NH * HD1],
                        start=False,
                        stop=True,
                    )
                # copy data cols only (ones cols already set)
                nc.scalar.copy(
                    v1view[:, i, :, 0:HD],
                    ps.rearrange("p (h c) -> p h c", c=HD1)[:, :, 0:HD],
                )

            # ---- stages 3+4: attention + output projection, per n-block ----
            psp.release()
            et_pool = tc.alloc_tile_pool(name="et", bufs=7)
            rc_pool = tc.alloc_tile_pool(name="rc", bufs=2)
            bc_pool = tc.alloc_tile_pool(name="bc", bufs=2)
            out_pool = tc.alloc_tile_pool(name="out", bufs=3)
            ps_s = tc.alloc_tile_pool(name="pss", bufs=2, space="PSUM")
            ps_av = tc.alloc_tile_pool(name="psav", bufs=1, space="PSUM")
            ps_o = tc.alloc_tile_pool(name="pso", bufs=1, space="PSUM")

            for n0, nw in BLOCKS:
                for h in range(NH):
                    ht, hp = divmod(h, 2)
                    qh = Q[ht][hp * HD : (hp + 1) * HD, n0 : n0 + nw]
                    ET = []  # (et_tile, col_offset) per m-chunk
                    for g in range(NG):
                        # 3 m-chunks per group, 512-aligned slots (bank-safe)
                        ps = ps_s.tile([128, 1536], F32, tag="s", name="s")
                        et = et_pool.tile([128, 1536], BF16, tag="et", name="et")
                        for j in range(3):
                            mc = g * 3 + j
                            kh = K[ht][
                                hp * HD : (hp + 1) * HD, mc * 128 : (mc + 1) * 128
                            ]
                            nc.tensor.matmul(
                                ps[:, j * 512 : j * 512 + nw],
                                kh,
                                qh,
                                start=True,
                                stop=True,
                            )
                            ET.append((et, j * 512))
                        psv_ = ps.rearrange("p (g c) -> p g c", c=512)[:, 0:3, 0:nw]
                        if ASSIGN[g] == "A":
                            etv_ = et.rearrange("p (g c) -> p g c", c=512)[:, 0:3, 0:nw]
                            nc.scalar.activation(etv_, psv_, AF.Exp, scale=0.125)
                        else:
                            eti_ = et.bitcast(I16).rearrange(
                                "p (g c) -> p g c", c=512
                            )[:, 0:3, 0:nw]
                            nc.vector.tensor_scalar(
                                eti_, psv_, _SCH_A, _SCH_B, ALU.mult, ALU.add
                            )
                    psa = ps_av.tile([HD1, NW], F32, tag="av", name="av")
                    for mc in range(NMC):
                        et, off = ET[mc]
                        nc.tensor.matmul(
                            psa[:, :nw],
                            v1view[:, mc, h, :],
                            et[:, off : off + nw],
                            start=(mc == 0),
                            stop=(mc == NMC - 1),
                        )
                    # normalize: sums row -> SBUF, K=1 PE broadcast into the
                    # shared o/bc PSUM bank, reciprocal, multiply on the
                    # PSUM->SBUF evacuation.
                    rc = rc_pool.tile([128, NW], F32R, tag="rc", name="rc")
                    nc.vector.tensor_copy(rc[HD : HD + 1, :nw], psa[HD : HD + 1, :nw])
                    psb = ps_o.tile([128, NW], F32, tag="o", name="o")
                    nc.tensor.matmul(
                        psb[0:HD, :nw],
                        ONES[HD : HD + 1, 0:HD],
                        rc[HD : HD + 1, :nw],
                        start=True,
                        stop=True,
                    )
                    bc = bc_pool.tile([HD, NW], F32, tag="bcs", name="bcs")
                    nc.vector.reciprocal(bc[:, :nw], psb[0:HD, :nw])
                    nc.vector.tensor_mul(
                        OH[h][0:HD, n0 : n0 + nw], psa[0:HD, :nw], bc[:, :nw]
                    )
                # ---- output projection for this n-block + fused residual ----
                for ot in range(2):
                    pso = ps_o.tile([128, NW], F32, tag="o", name="o")
                    for h in range(NH):
                        nc.tensor.matmul(
                            pso[:, :nw],
                            WO2[0:HD, h * C + ot * 128 : h * C + ot * 128 + 128],
                            OH[h][0:HD, n0 : n0 + nw],
                            start=(h == 0),
                            stop=(h == NH - 1) and not has_bo,
                        )
                    if has_bo:
                        nc.tensor.matmul(
                            pso[:, :nw],
                            BROW[0:1, NH * HD1 + ot * 128 : NH * HD1 + ot * 128 + 128],
                            ONES[0:1, 0:nw],
                            start=False,
                            stop=True,
                        )
                    outt = out_pool.tile([128, NW], F32, tag="out", name="out")
                    nc.vector.scalar_tensor_tensor(
                        outt[:, :nw],
                        pso[:, :nw],
                        GAMMA,
                        X[ot][:, n0 : n0 + nw].bitcast(F32),
                        op0=mybir.AluOpType.mult,
                        op1=mybir.AluOpType.add,
                    )
                    nc.sync.dma_start(
                        out=y_d[ot * 128 : (ot + 1) * 128, n0 : n0 + nw],
                        in_=outt[:, :nw],
                    )

            for p in (ps_o, ps_av, ps_s, out_pool, bc_pool, rc_pool, et_pool):
                p.release()
        consts.release()

    if for_hw:
        # walrus-compat rewrites; CoreSim can't execute post-hoc instructions
        _fix_unsupported_isa(nc)
        _split_multi_waits(nc)
    return nc


def make_in_maps(x, Wq, bq, Wk, bk, Wv, bv, Wo, bo, gamma):
    x = np.asarray(x, dtype=np.float32)
    B = x.shape[0]
    gamma = np.asarray(gamma, dtype=np.float32).reshape(-1)[0]
    f = lambda a: np.asarray(a, np.float32)

    wall = np.zeros((128, _WALL_W), np.float32)
    wqt, wkt, wvt = f(Wq).T, f(Wk).T, f(Wv).T
    wall[:, _WQ0 : _WQ0 + 256] = wqt[0:128]
    wall[:, _WQ1 : _WQ1 + 256] = wqt[128:256]
    wall[:, _WK0 : _WK0 + 256] = wkt[0:128]
    wall[:, _WK1 : _WK1 + 256] = wkt[128:256]
    wvt_aug = np.zeros((C, NH * HD1), np.float32)
    for h in range(NH):
        wvt_aug[:, h * HD1 : h * HD1 + HD] = wvt[:, h * HD : (h + 1) * HD]
    wall[:, _WV0 : _WV0 + NH * HD1] = wvt_aug[0:128]
    wall[:, _WV1 : _WV1 + NH * HD1] = wvt_aug[128:256]

    wot = f(Wo).T  # [c, o]
    wo2 = np.zeros((HD, NH * C), np.float32)
    for h in range(NH):
        wo2[:, h * C : (h + 1) * C] = wot[h * HD : (h + 1) * HD, :]

    misc = np.ones((128, 513), np.float32)
    misc[:, 0] = gamma

    bv_arr, bo_arr = f(bv).reshape(C), f(bo).reshape(C)
    brow = np.zeros((1, HD1 * NH + C), np.float32)
    for h in range(NH):
        brow[0, h * HD1 : h * HD1 + HD] = bv_arr[h * HD : (h + 1) * HD]
    brow[0, NH * HD1 :] = bo_arr

    bqk = np.stack([f(bq).reshape(C), f(bk).reshape(C)], axis=1)

    common = {
        "wall": wall,
        "wo2": wo2,
        "misc": misc,
        "v1ones": np.ones((128, NMC * NH), __import__("ml_dtypes").bfloat16),
        "bqk": bqk,
        "brow": brow,
    }
    flags = {
        "has_bqk": bool(np.any(bqk)),
        "has_bv": bool(np.any(bv_arr)),
        "has_bo": bool(np.any(bo_arr)),
    }
    in_maps = [
        {"x": np.ascontiguousarray(x[b].reshape(C, -1)), **common} for b in range(B)
    ]
    return in_maps, flags


_NC_CACHE = {}


def kernel(x, Wq, bq, Wk, bk, Wv, bv, Wo, bo, gamma):
    from concourse.bass_utils import run_bass_kernel_spmd

    x = np.asarray(x)
    B, Cc, H, W = x.shape
    in_maps, flags = make_in_maps(x, Wq, bq, Wk, bk, Wv, bv, Wo, bo, gamma)
    key = tuple(sorted(flags.items()))
    if key not in _NC_CACHE:
        _NC_CACHE[key] = build_module(**flags)
    res = run_bass_kernel_spmd(
        _NC_CACHE[key], in_maps, core_ids=list(range(len(in_maps)))
    )
    y = np.stack([res.results[b]["y"].reshape(Cc, H, W) for b in range(B)])
    return y.astype(x.dtype)
